# revision 13
# baseline (speedup 1.0000x reference)
"""MLA (multi-latent attention) prefill kernel for Trainium2, 8 NeuronCores.

Tensor-parallel over heads: each of the 8 cores owns 2 of the 16 heads.
w_q / w_kv_b are column-sharded, w_o row-sharded; the small kv_a latent
projection is replicated. Per-core partial outputs are summed on the host
(the "all-reduce" of the o_proj).

v2 dataflow (per core, column-major [feature, seq] layouts):
  qT   = wq_mod.T  @ hT     [256, S]  (2 M-blocks: h0[nope|x'], h1[nope|x'])
  kvaT = wkva_mod.T @ hT    [576, S]  (latent 512, then k-x')
  RoPE: x' = interleaved pe cols (folded into weights); the rotate-half
  operand y is a signed partition-shifted copy of x', built on the DVE
  (t2[0:32] = x'[32:64] * -sin, t2[32:64] = x'[0:32] * sin) instead of the
  extra "y" weight blocks -- saves ~20us of PE per kernel.
  rmsnorm over the latent 512 (partition dim): square on GpSimd, 4-block
  reduce on DVE, partition all-reduce on GpSimd, then
  rsqrt = Exp(-0.5 * Ln(mean + eps)) on ACT -- Ln/Exp/Copy/Square share one
  activation table so the kernel does a single table load total.
  The norm scale is folded into the bf16 cast of the latent (ckv), so
  kv_b consumes normalized latent directly.
  Attention is computed TRANSPOSED: scoresT[k, q] = kT_blk^T @ qT so the
  exp (ACT) writes probsT straight to SBUF -- no PE transposes and no
  PSUM->SBUF prob copies. Causal masking = gpsimd.affine_select zeroing
  invalid probs on the 4 diagonal key-blocks per superblock. Softmax sums
  come from a ones-row matmul accumulated in PSUM [1, 512]; normalization
  is deferred to the attnT epilogue (reciprocal + partition_broadcast).
  o = attnT.T @ w_o chunks, both heads accumulated in PSUM, streamed out.

Matmul operands are bf16 (full-rate PE, fp32 PSUM accumulation); softmax
statistics and rmsnorm statistics stay fp32.
"""
import os
import sys
import types
import numpy as np
import ml_dtypes

import concourse.bass as bass
import concourse.mybir as mybir
import concourse.tile as tile
from concourse import bacc, bass_isa, bass_utils, masks

F32 = mybir.dt.float32
BF16 = mybir.dt.bfloat16
F8 = mybir.dt.float8e4

S, HID = 2048, 2048
H, NOPE, ROPE, VD, KLR = 16, 64, 64, 128, 512
QD = NOPE + ROPE          # 128
SCALE = QD ** -0.5
EPS = 1e-6
NCORES = 8
HPC = H // NCORES         # heads per core = 2

SC = 512                  # seq chunk for projections
NSC = S // SC             # 4
HC = HID // 128           # 16 hid chunks
EXPB = 20.0               # fixed exp bias (overflow headroom)
WS = 64.0                 # fp8 weight pre-scale (folded back downstream)


def build_nc():
    nc = bacc.Bacc("TRN2", target_bir_lowering=False, debug=False,
                   num_devices=NCORES)
    dr = {}
    dr["hT"] = nc.dram_tensor("hT", [HID, S], BF16, kind="ExternalInput")
    dr["wq"] = nc.dram_tensor("wq", [HID, 256], BF16, kind="ExternalInput")
    dr["wkva"] = nc.dram_tensor("wkva", [HID, 640], BF16, kind="ExternalInput")
    dr["wkvb"] = nc.dram_tensor("wkvb", [KLR, 384], BF16, kind="ExternalInput")
    dr["wo"] = nc.dram_tensor("wo", [HPC * VD, HID], BF16, kind="ExternalInput")
    dr["cosd"] = nc.dram_tensor("cosd", [128, S], BF16, kind="ExternalInput")
    dr["msind"] = nc.dram_tensor("msind", [128, S], BF16, kind="ExternalInput")
    dr["o"] = nc.dram_tensor("o", [S, HID], BF16, kind="ExternalOutput")

    with tile.TileContext(nc) as tc:
        build_tile_kernel(nc, tc, {k: v.ap() for k, v in dr.items()})
    nc.compile()
    return nc


def build_tile_kernel(nc, tc, d):
    from contextlib import ExitStack
    with ExitStack() as ctx:
        _build_tile_kernel(nc, tc, d, ctx)


def _build_tile_kernel(nc, tc, d, ctx):
    AF = mybir.ActivationFunctionType
    ALU = mybir.AluOpType

    consts = ctx.enter_context(tc.tile_pool(name="consts", bufs=1))
    big = ctx.enter_context(tc.tile_pool(name="big", bufs=1))
    work = ctx.enter_context(tc.tile_pool(name="work", bufs=2))
    stat = ctx.enter_context(tc.tile_pool(name="stat", bufs=2))
    outp = ctx.enter_context(tc.tile_pool(name="outp", bufs=2))
    ps = ctx.enter_context(tc.tile_pool(name="ps", bufs=8, space="PSUM"))

    # ---- input DMAs ------------------------------------------------------
    # full hT resident in SBUF (32KB/partition); chunk-0 pieces land first
    hT_sb = consts.tile([128, HC, S], BF16)
    cos_sb = consts.tile([128, S], BF16)
    msin_sb = consts.tile([128, S], BF16)
    wkvb_sb = consts.tile([128, 4, 384], BF16)
    wo_sb = consts.tile([128, HPC, HID], BF16)
    for c in range(NSC):
        cs = slice(c * SC, (c + 1) * SC)
        for kp in range(HC // 2):
            nc.sync.dma_start(
                out=hT_sb[:, 2 * kp:2 * kp + 2, cs],
                in_=d["hT"][256 * kp:256 * (kp + 1), cs].rearrange(
                    "(k p) m -> p k m", p=128))
        if c == 0:
            nc.sync.dma_start(out=cos_sb[:], in_=d["cosd"])
            nc.sync.dma_start(out=msin_sb[:], in_=d["msind"])
            nc.sync.dma_start(out=wkvb_sb[:],
                              in_=d["wkvb"].rearrange("(k p) m -> p k m", p=128))
    wq_sb = consts.tile([128, HC, 256], BF16)
    wkva_sb = consts.tile([128, HC, 640], BF16)
    wq_r = d["wq"].rearrange("(k p) m -> p k m", p=128)
    wkva_r = d["wkva"].rearrange("(k p) m -> p k m", p=128)
    for k in range(HC):
        nc.scalar.dma_start(out=wq_sb[:, k, :], in_=wq_r[:, k, :])
        nc.scalar.dma_start(out=wkva_sb[:, k, :], in_=wkva_r[:, k, :])
    nc.sync.dma_start(out=wo_sb[:], in_=d["wo"].rearrange("(h p) n -> p h n", p=128))

    ones_bf = consts.tile([128, 128], BF16)
    nc.vector.memset(ones_bf[:], 1.0)
    one_f32 = consts.tile([1, 1], F32)
    nc.vector.memset(one_f32[:], 1.0)
    ident = consts.tile([128, 128], BF16)
    masks.make_identity(nc, ident[:])
    nexpb_sb = consts.tile([128, 1], F32)
    nc.vector.memset(nexpb_sb[:], -EXPB)
    eps_sb = consts.tile([128, 1], F32)
    nc.vector.memset(eps_sb[:], EPS)

    # ---- persistent activations -----------------------------------------
    qT = [big.tile([128, S], BF16, tag=f"qT{h}", name=f"qT{h}") for h in range(HPC)]
    kT = [big.tile([128, S], BF16, tag=f"kT{h}", name=f"kT{h}") for h in range(HPC)]
    v_sb = big.tile([128, S // 128, HPC * VD], BF16, tag="v")

    # =====================================================================
    def proj_mm1(c):
        """q/kpe wave + rope, latent wave + rmsnorm stats (part A)."""
        cs = slice(c * SC, (c + 1) * SC)
        # ---- wave 1: q blocks + shared k_pe x' block (3 PSUM banks) ----
        pq = [ps.tile([128, SC], F32, tag="ps", name=f"pq{i}") for i in range(HPC)]
        pkpe = ps.tile([128, SC], F32, tag="ps", name="pkpe")
        for k in range(HC):
            for h in range(HPC):
                nc.tensor.matmul(pq[h][:], wq_sb[:, k, h * 128:(h + 1) * 128],
                                 hT_sb[:, k, cs], start=(k == 0),
                                 stop=(k == HC - 1))
            nc.tensor.matmul(pkpe[:], wkva_sb[:, k, 512:640],
                             hT_sb[:, k, cs], start=(k == 0),
                             stop=(k == HC - 1))
        # rope epilogue: q' = x'*cos + y*sin, y = signed rotate-half of x'
        for h in range(HPC):
            nc.vector.tensor_copy(qT[h][0:64, cs], pq[h][0:64, :])
            t2 = work.tile([128, SC], F32, tag="t2", bufs=4)
            t3 = work.tile([128, SC], F32, tag="t2", bufs=4)
            nc.vector.tensor_tensor(t2[64:96, :], pq[h][96:128, :],
                                    msin_sb[96:128, cs], ALU.mult)
            nc.vector.tensor_tensor(t2[96:128, :], pq[h][64:96, :],
                                    msin_sb[64:96, cs], ALU.mult)
            nc.vector.tensor_tensor(t3[64:128, :], pq[h][64:128, :],
                                    cos_sb[64:128, cs], ALU.mult)
            nc.gpsimd.tensor_tensor(qT[h][64:128, cs], t3[64:128, :],
                                    t2[64:128, :], ALU.add)
        tk = work.tile([128, SC], F32, tag="t2", bufs=4)
        tk3 = work.tile([128, SC], F32, tag="t2", bufs=4)
        nc.vector.tensor_tensor(tk[0:32, :], pkpe[32:64, :],
                                msin_sb[32:64, cs], ALU.mult)
        nc.vector.tensor_tensor(tk[32:64, :], pkpe[0:32, :],
                                msin_sb[0:32, cs], ALU.mult)
        nc.vector.tensor_tensor(tk3[0:64, :], pkpe[0:64, :],
                                cos_sb[0:64, cs], ALU.mult)
        nc.gpsimd.tensor_tensor(kT[0][64:128, cs], tk3[0:64, :],
                                tk[0:64, :], ALU.add)
        nc.vector.tensor_copy(kT[1][64:128, cs], kT[0][64:128, cs])

        # ---- wave 2: latent blocks (4 PSUM banks) ----
        plat = [ps.tile([128, SC], F32, tag="ps", name=f"plat{i}") for i in range(4)]
        for k in range(HC):
            for m in range(4):
                nc.tensor.matmul(plat[m][:], wkva_sb[:, k, m * 128:(m + 1) * 128],
                                 hT_sb[:, k, cs], start=(k == 0),
                                 stop=(k == HC - 1))
        # stats part A: stage, square (gpsimd), 4-block sum, partition sum
        stg = work.tile([128, 4, SC], F32, tag="stg", bufs=1)
        nc.vector.tensor_copy(stg[:, 0, :], plat[0][:])
        nc.vector.tensor_copy(stg[:, 1, :], plat[1][:])
        nc.scalar.copy(stg[:, 2, :], plat[2][:])
        nc.scalar.copy(stg[:, 3, :], plat[3][:])
        sq = work.tile([128, 4, SC], BF16, tag="sq", bufs=1)
        nc.gpsimd.tensor_tensor(sq[:], stg[:], stg[:], ALU.mult)
        return stg, sq

    def stats_b(stg, sq):
        """Sum of squares over the latent dim via a ones-matmul on the PE
        (replaces reduce_sum + partition_all_reduce and their cross-engine
        serialization), then rsqrt via Ln+Exp (same ACT table as the
        attention Exp), then fold the norm scale into the bf16 latent
        cast."""
        pssq = ps.tile([128, SC], F32, tag="ps", name="pssq")
        for m in range(4):
            nc.tensor.matmul(pssq[:], ones_bf[:], sq[:, m, :],
                             start=(m == 0), stop=(m == 3))
        lnm = work.tile([128, SC], F32, tag="lnm", bufs=1)
        nc.scalar.activation(lnm[:], pssq[:], AF.Ln, bias=eps_sb[:],
                             scale=1.0 / KLR)
        sbc = work.tile([128, SC], F32, tag="sbc", bufs=1)
        nc.scalar.activation(sbc[:], lnm[:], AF.Exp, scale=-0.5)
        ckv = work.tile([128, 4, SC], BF16, tag="ckv", bufs=1)
        for m in range(4):
            nc.vector.tensor_tensor(ckv[:, m, :], stg[:, m, :], sbc[:],
                                    ALU.mult)
        return ckv

    def proj_mm2(c, ckv):
        """kv_b: k_nope (column-major) and v (row-major) from normalized
        latent."""
        cs = slice(c * SC, (c + 1) * SC)
        pnope = ps.tile([128, SC], F32, tag="ps", name="pnope")
        for kk in range(4):
            nc.tensor.matmul(pnope[:], wkvb_sb[:, kk, 0:128], ckv[:, kk, :],
                             start=(kk == 0), stop=(kk == 3))
        nc.vector.tensor_copy(kT[0][0:64, cs], pnope[0:64, :])
        nc.vector.tensor_copy(kT[1][0:64, cs], pnope[64:128, :])
        for t in range(4):
            pv = ps.tile([128, HPC * VD], F32, tag="ps", name="pv")
            for kk in range(4):
                nc.tensor.matmul(pv[:], ckv[:, kk, t * 128:(t + 1) * 128],
                                 wkvb_sb[:, kk, 128:384],
                                 start=(kk == 0), stop=(kk == 3))
            nc.vector.tensor_copy(v_sb[:, 4 * c + t, :], pv[:])

    # =====================================================================
    def attn_core(B, mid=None):
        """Transposed-scores attention for superblock B (512 queries), both
        heads.  Per key-block kt: scoresT (PE) -> exp (ACT, writes probsT to
        SBUF) -> [causal zero via affine_select on diagonal blocks (gpsimd)]
        -> attnT accumulate + ones sum accumulate (PE).  The 1/sumexp scale
        is folded into the PSUM->SBUF copy of attnT; its reciprocal chain
        (transpose->recip->transpose->broadcast) is scheduled after both
        heads so the PE never waits on it."""
        nkt = 4 * (B + 1)
        LAG = 2
        pa = [None, None]
        pone = [None, None]
        serow = [None, None]
        at = [None, None]
        for h in range(HPC):
            pa[h] = ps.tile([128, 512], F32, tag="ps", name=f"pa{h}")
            pone[h] = ps.tile([128, 512], F32, tag="ps", name=f"pone{h}")
            pts = {}
            for step in range(nkt + LAG):
                if step < nkt:
                    kt = step
                    qoff = max(0, (kt - 4 * B) * 128)
                    psc = ps.tile([128, 512], F32, tag="ps", name="psc")
                    nc.tensor.matmul(
                        psc[:, qoff:512],
                        kT[h][:, kt * 128:(kt + 1) * 128],
                        qT[h][:, B * 512 + qoff:(B + 1) * 512],
                        start=True, stop=True)
                    pt = work.tile([128, 512], BF16, tag="pt", bufs=4,
                                   name="pt")
                    nc.scalar.activation(pt[:, qoff:512], psc[:, qoff:512],
                                         AF.Exp, bias=nexpb_sb[:], scale=1.0)
                    if kt >= 4 * B:
                        # zero probs where query < key (incl. stale cols)
                        nc.gpsimd.affine_select(
                            out=pt[:], in_=pt[:], compare_op=ALU.is_ge,
                            fill=0.0, base=B * 512 - kt * 128,
                            channel_multiplier=-1, pattern=[[1, 512]])
                    pts[kt] = pt
                if step >= LAG:
                    kt = step - LAG
                    pt = pts.pop(kt)
                    qo = max(0, (kt - 4 * B) * 128)
                    nc.tensor.matmul(pa[h][:, qo:512],
                                     v_sb[:, kt, h * VD:(h + 1) * VD],
                                     pt[:, qo:512], start=(kt == 0),
                                     stop=(kt == nkt - 1))
                    nc.tensor.matmul(pone[h][:, qo:512], ones_bf[:],
                                     pt[:, qo:512], start=(kt == 0),
                                     stop=(kt == nkt - 1))
            serow[h] = stat.tile([1, 512], F32, tag="serow", name="serow")
            nc.vector.tensor_copy(serow[h][:], pone[h][0:1, :])
            if h == 0 and mid is not None:
                mid()
        # ---- normalization epilogue for both heads ----
        pcol = [None, None]
        for h in range(HPC):
            pcol[h] = ps.tile([128, 4], F32, tag="ps", name=f"pcol{h}")
            for qi in range(4):
                nc.tensor.transpose(pcol[h][:, qi:qi + 1],
                                    serow[h][0:1, qi * 128:(qi + 1) * 128],
                                    one_f32[0:1, 0:1])
        rb4 = [None, None]
        for h in range(HPC):
            r4 = stat.tile([128, 4], F32, tag="r4", name="r4")
            nc.vector.reciprocal(r4[:], pcol[h][:])
            rb4[h] = stat.tile([128, 4], BF16, tag="rb4", name="rb4")
            nc.vector.tensor_copy(rb4[h][:], r4[:])
        prt = [None, None]
        for h in range(HPC):
            prt[h] = ps.tile([1, 512], BF16, tag="ps", name=f"prt{h}")
            for qi in range(4):
                nc.tensor.transpose(prt[h][0:1, qi * 128:(qi + 1) * 128],
                                    rb4[h][:, qi:qi + 1], ident[:])
        for h in range(HPC):
            rrbf = stat.tile([1, 512], BF16, tag="rrbf", name="rrbf")
            nc.vector.tensor_copy(rrbf[:], prt[h][0:1, :])
            rbc = work.tile([128, 512], BF16, tag="rbc")
            nc.gpsimd.partition_broadcast(rbc[:], rrbf[0:1, :])
            a = work.tile([128, 512], BF16, tag=f"at{h}", name=f"at{h}")
            nc.vector.tensor_tensor(a[:], pa[h][:], rbc[:], ALU.mult)
            at[h] = a
        return at

    def attn_oproj(B, at):
        for t in range(4):
            ot = outp.tile([128, 4, 512], BF16, tag="ot")
            for n in range(4):
                po = ps.tile([128, 512], F32, tag="ps", name="po")
                for h in range(HPC):
                    nc.tensor.matmul(po[:], at[h][:, t * 128:(t + 1) * 128],
                                     wo_sb[:, h, n * 512:(n + 1) * 512],
                                     start=(h == 0), stop=(h == HPC - 1))
                if n % 2 == 0:
                    nc.vector.tensor_copy(ot[:, n, :], po[:])
                else:
                    nc.scalar.copy(ot[:, n, :], po[:])
            nc.sync.dma_start(
                out=d["o"][(4 * B + t) * 128:(4 * B + t + 1) * 128, :],
                in_=ot[:])

    # =====================================================================
    stgssq = [None] * NSC
    stgssq[0] = proj_mm1(0)
    stgssq[1] = proj_mm1(1)
    proj_mm2(0, stats_b(*stgssq[0]))
    for c in range(1, NSC):
        holder = {}

        def mid(pair=stgssq[c], holder=holder):
            holder["ckv"] = stats_b(*pair)

        at = attn_core(c - 1, mid=mid)
        proj_mm2(c, holder["ckv"])
        if c + 1 < NSC:
            stgssq[c + 1] = proj_mm1(c + 1)
        attn_oproj(c - 1, at)
    at = attn_core(NSC - 1)
    attn_oproj(NSC - 1, at)


# =========================================================================
# host side
# =========================================================================
_perm1 = np.concatenate([np.arange(0, ROPE, 2), np.arange(1, ROPE, 2)])


def _host_prep(inputs):
    hidden = np.ascontiguousarray(np.asarray(inputs["hidden_states"],
                                             dtype=np.float32)[0])
    cos = np.asarray(inputs["cos"], dtype=np.float32)[0]
    sin = np.asarray(inputs["sin"], dtype=np.float32)[0]
    w_q = np.asarray(inputs["w_q"], dtype=np.float32)
    w_kv_a = np.asarray(inputs["w_kv_a"], dtype=np.float32)
    ln_w = np.asarray(inputs["kv_a_ln_w"], dtype=np.float32)
    w_kv_b = np.asarray(inputs["w_kv_b"], dtype=np.float32)
    w_o = np.asarray(inputs["w_o"], dtype=np.float32)

    hT = np.ascontiguousarray(hidden.T)
    cosT = cos.T
    sinT = sin.T
    # cos rows duplicated: rows 0:64 for kpe (psum parts 0:64), 64:128 for q
    cosd = np.ascontiguousarray(np.concatenate([cosT, cosT], axis=0))
    # msin rows placed at the partitions of the x' operand they multiply:
    #  0:32  -> sin[32:64]  (kpe t2b reads pkpe[0:32])
    #  32:64 -> -sin[0:32]  (kpe t2a reads pkpe[32:64])
    #  64:96 -> sin[32:64]  (q t2b reads pq[64:96])
    #  96:128-> -sin[0:32]  (q t2a reads pq[96:128])
    msind = np.ascontiguousarray(np.concatenate(
        [sinT[32:64], -sinT[0:32], sinT[32:64], -sinT[0:32]], axis=0))

    kpe_cols = w_kv_a[:, KLR:]
    kpe_x = kpe_cols[:, _perm1]
    wkva_mod = np.ascontiguousarray(np.concatenate(
        [w_kv_a[:, :KLR], kpe_x, kpe_x], axis=1))            # [HID, 640]
    wkvb_all = w_kv_b * ln_w[:, None]

    bf = ml_dtypes.bfloat16
    in_maps = []
    for cid in range(NCORES):
        heads = [HPC * cid + i for i in range(HPC)]
        blocks = []
        for h in heads:
            wq_h = w_q[:, h * QD:(h + 1) * QD]
            blocks.append(np.concatenate(
                [wq_h[:, :NOPE], wq_h[:, NOPE:][:, _perm1]], axis=1))
        wq_mod = np.ascontiguousarray(np.concatenate(blocks, axis=1) * SCALE)

        nope_b = [wkvb_all[:, h * (NOPE + VD):h * (NOPE + VD) + NOPE]
                  for h in heads]
        v_b = [wkvb_all[:, h * (NOPE + VD) + NOPE:(h + 1) * (NOPE + VD)]
               for h in heads]
        wkvb_mod = np.ascontiguousarray(np.concatenate(nope_b + v_b, axis=1))

        wo_mod = np.ascontiguousarray(w_o[heads[0] * VD:(heads[-1] + 1) * VD, :])

        in_maps.append({"hT": hT.astype(bf), "wq": wq_mod.astype(bf),
                        "wkva": wkva_mod.astype(bf),
                        "wkvb": wkvb_mod.astype(bf), "wo": wo_mod.astype(bf),
                        "cosd": cosd.astype(bf), "msind": msind.astype(bf)})
    return in_maps


def _install_ntff_hook():
    """Make trace=True work under axon (antenv.axon_hooks is absent in this
    image; back it with trn_agent_boot's ctypes hook)."""
    try:
        import antenv
        if "antenv.axon_hooks" in sys.modules:
            return
        from trn_agent_boot.trn_boot import _ntff_profile_via_ctypes
        hook = _ntff_profile_via_ctypes("/opt/axon/libaxon_pjrt.so")
        mod = types.ModuleType("antenv.axon_hooks")
        mod.get_axon_ntff_profile_hook = lambda: hook
        mod.set_axon_ntff_profile_hook = lambda h: None
        sys.modules["antenv.axon_hooks"] = mod
        antenv.axon_hooks = mod
    except Exception:
        pass


_nc_cache = None
last_results = None


def kernel(**inputs):
    global _nc_cache, last_results
    _install_ntff_hook()
    if _nc_cache is None:
        _nc_cache = build_nc()
    in_maps = _host_prep(inputs)
    trace = bool(os.environ.get("BASS_TRACE"))
    res = bass_utils.run_bass_kernel_spmd(
        _nc_cache, in_maps, core_ids=list(range(NCORES)), trace=trace)
    last_results = res
    total = res.results[0]["o"].astype(np.float32)
    for c in range(1, NCORES):
        total = total + res.results[c]["o"]
    return total.reshape(1, S, HID)
